# revision 1
# baseline (speedup 1.0000x reference)
"""Trainium2 Bass kernel for nn_Conv3DSynthesisLayer.

Computes, per sample b (one NeuronCore each, data-parallel over batch B=8):
  styles = w[b] @ (affine_weight / sqrt(512)).T + affine_bias        [Cin]
  wmod   = weight * styles[None,:,None..] ; demod by rsqrt(sumsq)    [Cout,Cin,3,3,3]
  out    = lrelu(conv3d(x[b], wmod, pad=1) + bias) * sqrt(2)         [Cout,32,32,32]

Implementation notes (v2, bf16):
  - Conv is 27 shifted bf16 matmuls (K=Cin=128 on partitions) accumulated in
    PSUM per output d-slice, over an h/w zero-padded x laid out per-slice as
    [128, 34 rows x 64-elem pitch] in SBUF (pitch 64 keeps row starts 128B
    aligned).  D-boundary taps are skipped (no d padding).
  - bf16 matmuls issue at ~228ns/512 rows vs fp32r's ~241ns (hw-measured);
    rounding error lands at rel ~2.6e-3, far inside the 2e-2 gate.
  - Modulated bf16 weights are built in one DVE op per tap straight from the
    PE-transposed PSUM tile; the demod sum-of-squares uses Square on the
    Scalar engine + a styles^2 ones-matmul, freeing the DVE for x placement.
  - Startup is reordered so x block placement and the styles chain run before
    the per-tap weight build; the conv starts ~9us in instead of ~35us.
  - Demodulation and the lrelu epilogue stay fused in one ScalarE Prelu per
    PSUM bank: out = prelu(psum*scale + bias*g).
"""
import sys

sys.path.insert(0, "/opt/trn_rl_repo")

import numpy as np
from contextlib import ExitStack

import concourse.mybir as mybir
import concourse.tile as tile
from concourse import bacc
from concourse.masks import make_identity
from concourse import bass_utils as _bass_utils
from concourse.bass_utils import run_bass_kernel_spmd

# Walrus's ldw-opt pass rejects the explicit InstLdweights that bf16 matmuls
# emit ("not compatible with LDW optimization"), so it must stay off; the
# per-matmul bf16 weight loads pipeline under the previous matmul for free
# (hw-measured: alternating-stationary bf16 runs at the same 216ns/512rows).
_LDW_OPT = False
if not getattr(_bass_utils, "_ldw_opt_patched", False):
    _orig_run_command = _bass_utils.run_command

    def _run_command_ldw(argv, **kw):
        if _LDW_OPT and isinstance(argv, (list, tuple)):
            argv = ["--enable-ldw-opt=true" if a == "--enable-ldw-opt=false" else a
                    for a in argv]
        return _orig_run_command(argv, **kw)

    _bass_utils.run_command = _run_command_ldw
    _bass_utils._ldw_opt_patched = True

F32 = mybir.dt.float32
BF16 = mybir.dt.bfloat16
AF = mybir.ActivationFunctionType

B, CIN, COUT, R = 8, 128, 128, 32
W_DIM = 512
NTAPS = 27
RP = R + 2   # 34: h/w padded extent
PW = 64      # row pitch (elems) so bf16 rows start 128B-aligned
GAIN = float(np.sqrt(2.0).astype(np.float32))
SLOPE = 0.2
EPS = 1e-8
DBLK = 4     # d-slices per x block
NBLK = R // DBLK
GD = 2       # d-slices per psum group (taps-outer)
NCORES = 8

_cache = {}


def _build():
    nc = bacc.Bacc("TRN2", target_bir_lowering=False, debug=False, num_devices=NCORES)
    x_d = nc.dram_tensor("x", [CIN, R * R * R], F32, kind="ExternalInput").ap()
    wv_d = nc.dram_tensor("wvec", [W_DIM], F32, kind="ExternalInput").ap()
    wt_d = nc.dram_tensor("weight", [COUT, CIN * NTAPS], F32, kind="ExternalInput").ap()
    aw_d = nc.dram_tensor("aw", [CIN, W_DIM], F32, kind="ExternalInput").ap()
    ab_d = nc.dram_tensor("ab", [CIN], F32, kind="ExternalInput").ap()
    bs_d = nc.dram_tensor("bias", [COUT], F32, kind="ExternalInput").ap()
    out_d = nc.dram_tensor("out", [COUT, R * R * R], F32, kind="ExternalOutput").ap()

    ctx = ExitStack()
    with ctx:
        tc = ctx.enter_context(tile.TileContext(nc))
        singles = ctx.enter_context(tc.tile_pool(name="singles", bufs=1))
        xpool = ctx.enter_context(tc.tile_pool(name="xpool", bufs=4))
        stpool = ctx.enter_context(tc.tile_pool(name="stpool", bufs=2))
        sqpool = ctx.enter_context(tc.tile_pool(name="sqpool", bufs=3))
        obpool = ctx.enter_context(tc.tile_pool(name="obpool", bufs=4))

        # ---- param DMAs up front ----
        aw_sb = singles.tile([128, W_DIM], F32)
        nc.sync.dma_start(out=aw_sb, in_=aw_d)
        wv_sb = singles.tile([128, 4], F32)
        nc.sync.dma_start(out=wv_sb, in_=wv_d.rearrange("(c k) -> k c", k=128))
        ab_sb = singles.tile([128, 1], F32)
        nc.sync.dma_start(out=ab_sb, in_=ab_d.rearrange("(p one) -> p one", one=1))
        bs_sb = singles.tile([128, 1], F32)
        nc.sync.dma_start(out=bs_sb, in_=bs_d.rearrange("(p one) -> p one", one=1))
        wnat = singles.tile([128, CIN * NTAPS], F32)
        nc.sync.dma_start(out=wnat, in_=wt_d)

        # ---- x staging / padded-block machinery ----
        zrow = singles.tile([128, DBLK * PW], F32)
        nc.vector.memset(zrow, 0.0)
        zview = zrow.rearrange("p (d e) -> p d e", e=PW)

        x_r = x_d.rearrange("p (d hw) -> p d hw", hw=R * R)
        xblocks = [None] * NBLK

        def stage_block(blk):
            stag = stpool.tile([128, DBLK, R * R], F32, tag="stag", name=f"st{blk}")
            nc.sync.dma_start(out=stag, in_=x_r[:, blk * DBLK:(blk + 1) * DBLK, :])
            return stag

        def place_block(blk, stag):
            xb = xpool.tile([128, DBLK, RP, PW], BF16, tag="xb", name=f"xb{blk}")
            nc.vector.tensor_copy(out=xb[:, :, 0, 0:RP], in_=zview[:, :, 0:RP])
            nc.vector.tensor_copy(out=xb[:, :, RP - 1, 0:RP], in_=zview[:, :, 0:RP])
            nc.vector.tensor_copy(out=xb[:, :, :, 0], in_=zview[:, :, 0:RP])
            nc.vector.tensor_copy(out=xb[:, :, :, RP - 1], in_=zview[:, :, 0:RP])
            stv = stag.rearrange("p d (h w) -> p d h w", w=R)
            for si in range(DBLK):
                nc.vector.tensor_copy(out=xb[:, si, 1:R + 1, 1:R + 1],
                                      in_=stv[:, si, :, :])
            xblocks[blk] = xb

        st0 = stage_block(0)
        st1 = stage_block(1)

        # ---- phase A: styles, modulated bf16 weights, demod scale ----
        with tc.tile_pool(name="ps_a", bufs=2, space="PSUM") as ps_a:
            ident = singles.tile([128, 128], F32)
            make_identity(nc, ident)

            # affine_weight.T, chunked over the 512-dim
            awt = singles.tile([128, W_DIM], F32)
            for c in range(4):
                paw = ps_a.tile([128, 128], F32, tag="paw", name=f"paw{c}")
                nc.tensor.transpose(paw, aw_sb[:, c * 128:(c + 1) * 128], ident)
                nc.vector.tensor_copy(out=awt[:, c * 128:(c + 1) * 128], in_=paw)

            # styles[ci] = sum_k aw[ci,k] w[k] / sqrt(512) + ab[ci]
            ps_sty = ps_a.tile([128, 1], F32, tag="ps_sty")
            for c in range(4):
                nc.tensor.matmul(ps_sty, lhsT=awt[:, c * 128:(c + 1) * 128],
                                 rhs=wv_sb[:, c:c + 1], start=(c == 0), stop=(c == 3))
            styles = singles.tile([128, 1], F32)
            nc.scalar.activation(out=styles, in_=ps_sty, func=AF.Identity,
                                 bias=ab_sb, scale=1.0 / float(np.sqrt(W_DIM)))
            styles2 = singles.tile([128, 1], F32)
            nc.vector.tensor_mul(out=styles2, in0=styles, in1=styles)

            # x block 0 placed before the weight chain so conv can start early;
            # block 1 (first read ~33us in) placed after it so the phase-A PSUM
            # pool releases sooner (the conv's PSUM banks wait on that release).
            place_block(0, st0)

            # per-tap: transpose [co,ci]->[ci,co]; modulate to bf16 (DVE);
            # square on ScalarE; accumulate sumsq via styles^2-matmul (PE).
            wnat_t = wnat.rearrange("p (ci t) -> p t ci", t=NTAPS)
            w1b = singles.tile([128, NTAPS * 128], BF16)
            ps_dm = ps_a.tile([128, 1], F32, tag="ps_dm")
            for t in range(NTAPS):
                pw = ps_a.tile([128, 128], F32, tag="paw", name=f"pw{t}")
                nc.tensor.transpose(pw, wnat_t[:, t, :], ident)
                nc.vector.tensor_scalar_mul(out=w1b[:, t * 128:(t + 1) * 128],
                                            in0=pw, scalar1=styles)
                sq = sqpool.tile([128, 128], F32, tag="sq", name=f"sq{t}")
                nc.scalar.activation(out=sq, in_=pw, func=AF.Square)
                nc.tensor.matmul(ps_dm, lhsT=sq, rhs=styles2,
                                 start=(t == 0), stop=(t == NTAPS - 1))

            place_block(1, st1)

            # scale[co] = GAIN * rsqrt(sumsq+EPS) = 1/sqrt(sumsq/G^2 + EPS/G^2)
            eps_sb = singles.tile([128, 1], F32)
            nc.vector.memset(eps_sb, EPS / (GAIN * GAIN))
            sc_tmp = singles.tile([128, 1], F32)
            nc.scalar.activation(out=sc_tmp, in_=ps_dm, func=AF.Sqrt,
                                 bias=eps_sb, scale=1.0 / (GAIN * GAIN))
            scale_sb = singles.tile([128, 1], F32)
            nc.vector.reciprocal(out=scale_sb, in_=sc_tmp)
            bias_g = singles.tile([128, 1], F32)
            nc.scalar.mul(out=bias_g, in_=bs_sb, mul=GAIN)

        pspool = ctx.enter_context(tc.tile_pool(name="pspool", bufs=2, space="PSUM"))

        # ---- phase B: the conv ----
        next_blk = 2
        for g in range(R // GD):
            d0 = g * GD
            while next_blk < NBLK and (d0 + GD) // DBLK + 1 >= next_blk:
                place_block(next_blk, stage_block(next_blk))
                next_blk += 1

            ds = list(range(d0, d0 + GD))
            valid = {d: [kd for kd in range(3) if 0 <= d + kd - 1 < R] for d in ds}
            first_t = {d: min(v) * 9 for d, v in valid.items()}
            last_t = {d: max(v) * 9 + 8 for d, v in valid.items()}

            ps = {(dd, hh): pspool.tile([128, 512], F32, tag=f"ps{dd}{hh}",
                                        name=f"ps{d0}_{dd}{hh}")
                  for dd in range(GD) for hh in range(2)}

            for kd in range(3):
                for kh in range(3):
                    for kw in range(3):
                        t = kd * 9 + kh * 3 + kw
                        lhs = w1b[:, t * 128:(t + 1) * 128]
                        for dd, d in enumerate(ds):
                            s = d + kd - 1
                            if not (0 <= s < R):
                                continue
                            xb = xblocks[s // DBLK]
                            si = s % DBLK
                            for hh in range(2):
                                rhs = xb[:, si, 16 * hh + kh: 16 * hh + kh + 16,
                                         kw:kw + 32]
                                nc.tensor.matmul(
                                    ps[(dd, hh)], lhsT=lhs, rhs=rhs,
                                    start=(t == first_t[d]), stop=(t == last_t[d]))

            for dd, d in enumerate(ds):
                ob = obpool.tile([128, 1024], F32, tag="ob", name=f"ob{d}")
                for hh in range(2):
                    nc.scalar.activation(out=ob[:, hh * 512:(hh + 1) * 512],
                                         in_=ps[(dd, hh)], func=AF.Prelu,
                                         bias=bias_g, scale=scale_sb, alpha=SLOPE)
                nc.sync.dma_start(out=out_d[:, d * 1024:(d + 1) * 1024], in_=ob)

    nc.compile()
    return nc


def kernel(**inputs):
    x = np.ascontiguousarray(np.asarray(inputs["x"], dtype=np.float32))
    w = np.ascontiguousarray(np.asarray(inputs["w"], dtype=np.float32))
    weight = np.ascontiguousarray(np.asarray(inputs["weight"], dtype=np.float32))
    aw = np.ascontiguousarray(np.asarray(inputs["affine_weight"], dtype=np.float32))
    ab = np.ascontiguousarray(np.asarray(inputs["affine_bias"], dtype=np.float32))
    bias = np.ascontiguousarray(np.asarray(inputs["bias"], dtype=np.float32))

    if "nc" not in _cache:
        _cache["nc"] = _build()
    nc = _cache["nc"]

    wt2 = weight.reshape(COUT, CIN * NTAPS)
    in_maps = [
        {
            "x": x[b].reshape(CIN, R * R * R),
            "wvec": w[b],
            "weight": wt2,
            "aw": aw,
            "ab": ab,
            "bias": bias,
        }
        for b in range(B)
    ]
    res = run_bass_kernel_spmd(nc, in_maps, list(range(NCORES)))
    out = np.stack([res.results[b]["out"].reshape(COUT, R, R, R) for b in range(B)])
    return out.astype(np.float32)


def run_traced(**inputs):
    """Like kernel(), but also returns the profiled HW exec time in ns."""
    x = np.asarray(inputs["x"], dtype=np.float32)
    w = np.asarray(inputs["w"], dtype=np.float32)
    weight = np.asarray(inputs["weight"], dtype=np.float32)
    aw = np.asarray(inputs["affine_weight"], dtype=np.float32)
    ab = np.asarray(inputs["affine_bias"], dtype=np.float32)
    bias = np.asarray(inputs["bias"], dtype=np.float32)
    if "nc" not in _cache:
        _cache["nc"] = _build()
    nc = _cache["nc"]
    wt2 = weight.reshape(COUT, CIN * NTAPS)
    in_maps = [
        {"x": x[b].reshape(CIN, R * R * R), "wvec": w[b], "weight": wt2,
         "aw": aw, "ab": ab, "bias": bias}
        for b in range(B)
    ]
    res = run_bass_kernel_spmd(nc, in_maps, list(range(NCORES)), trace=True)
    out = np.stack([res.results[b]["out"].reshape(COUT, R, R, R) for b in range(B)])
    return out.astype(np.float32), res.exec_time_ns, res



# revision 7
# speedup vs baseline: 1.2724x; 1.2724x over previous
"""Trainium2 Bass kernel for nn_Conv3DSynthesisLayer.

Computes, per sample b (one NeuronCore each, data-parallel over batch B=8):
  styles = w[b] @ (affine_weight / sqrt(512)).T + affine_bias        [Cin]
  wmod   = weight * styles[None,:,None..] ; demod by rsqrt(sumsq)    [Cout,Cin,3,3,3]
  out    = lrelu(conv3d(x[b], wmod, pad=1) + bias) * sqrt(2)         [Cout,32,32,32]

v3: Winograd F(2,3) along the W axis, bf16.
  - The kw-dim of the conv is computed in the 4-point transform domain:
    X~[j,t] = B^T x over 6->4... F(2,3): per 2-output tile t (16 tiles/row),
    X~0 = x[2t-1]-x[2t+1], X~1 = x[2t]+x[2t+1], X~2 = x[2t+1]-x[2t],
    X~3 = x[2t]-x[2t+2]  (padded coords).  W~0 = w0, W~1 = (w0+w1+w2)/2,
    W~2 = (w0-w1+w2)/2, W~3 = w2 (modulated by styles).
  - Per output d-slice: 4 psum banks (one per j), each accumulating 9
    (kd,kh)-tap matmuls of full 512 rows (K=Cin=128 on partitions):
    36 matmuls/slice vs 54 for direct conv -> 1.5x less PE time.
  - Inverse transform (even w: m0+m1+m2, odd w: m1-m2-m3) on DVE/GpSimd
    from PSUM, then the fused Prelu epilogue (demod scale + bias + lrelu
    gain) on ScalarE with strided even/odd writes, then DMA per slice.
  - X~ is built directly from the raw staged x (no padded-x intermediate):
    6 strided DVE/GpSimd ops + 2 border memsets per 4-slice block.
"""
import sys

sys.path.insert(0, "/opt/trn_rl_repo")

import numpy as np
from contextlib import ExitStack

import concourse.mybir as mybir
import concourse.tile as tile
from concourse import bacc
from concourse.masks import make_identity
from concourse.bass_utils import run_bass_kernel_spmd

F32 = mybir.dt.float32
BF16 = mybir.dt.bfloat16
AF = mybir.ActivationFunctionType
ALU = mybir.AluOpType

B, CIN, COUT, R = 8, 128, 128, 32
W_DIM = 512
NTAPS = 27
HP = R + 2    # 34: padded h extent of X~
NJ = 4        # winograd transform points
NT = 16       # w-tiles per row (2 outputs each)
GAIN = float(np.sqrt(2.0).astype(np.float32))
SLOPE = 0.2
EPS = 1e-8
DBLK = 4      # d-slices per x block
NBLK = R // DBLK
NCORES = 8

_cache = {}


def _build():
    nc = bacc.Bacc("TRN2", target_bir_lowering=False, debug=False, num_devices=NCORES)
    x_d = nc.dram_tensor("x", [CIN, R * R * R], F32, kind="ExternalInput").ap()
    wv_d = nc.dram_tensor("wvec", [W_DIM], F32, kind="ExternalInput").ap()
    wt_d = nc.dram_tensor("weight", [COUT, CIN * NTAPS], F32, kind="ExternalInput").ap()
    aw_d = nc.dram_tensor("aw", [CIN, W_DIM], F32, kind="ExternalInput").ap()
    ab_d = nc.dram_tensor("ab", [CIN], F32, kind="ExternalInput").ap()
    bs_d = nc.dram_tensor("bias", [COUT], F32, kind="ExternalInput").ap()
    out_d = nc.dram_tensor("out", [COUT, R * R * R], F32, kind="ExternalOutput").ap()

    ctx = ExitStack()
    with ctx:
        tc = ctx.enter_context(tile.TileContext(nc))
        singles = ctx.enter_context(tc.tile_pool(name="singles", bufs=1))
        xpool = ctx.enter_context(tc.tile_pool(name="xpool", bufs=4))
        stpool = ctx.enter_context(tc.tile_pool(name="stpool", bufs=2))
        pwpool = ctx.enter_context(tc.tile_pool(name="pwpool", bufs=4))
        sqpool = ctx.enter_context(tc.tile_pool(name="sqpool", bufs=3))
        tpool = ctx.enter_context(tc.tile_pool(name="tpool", bufs=3))
        obpool = ctx.enter_context(tc.tile_pool(name="obpool", bufs=4))

        # ---- param DMAs up front ----
        aw_sb = singles.tile([128, W_DIM], F32)
        nc.sync.dma_start(out=aw_sb, in_=aw_d)
        wv_sb = singles.tile([128, 4], F32)
        nc.sync.dma_start(out=wv_sb, in_=wv_d.rearrange("(c k) -> k c", k=128))
        ab_sb = singles.tile([128, 1], F32)
        nc.sync.dma_start(out=ab_sb, in_=ab_d.rearrange("(p one) -> p one", one=1))
        bs_sb = singles.tile([128, 1], F32)
        nc.sync.dma_start(out=bs_sb, in_=bs_d.rearrange("(p one) -> p one", one=1))
        wnat = singles.tile([128, CIN * NTAPS], F32)
        nc.sync.dma_start(out=wnat, in_=wt_d)

        # ---- x staging + winograd input transform ----
        x_r = x_d.rearrange("p (d hw) -> p d hw", hw=R * R)
        xblocks = [None] * NBLK

        def stage_block(blk):
            stag = stpool.tile([128, DBLK, R, R], F32, tag="stag", name=f"st{blk}")
            nc.sync.dma_start(
                out=stag.rearrange("p d h w -> p d (h w)"),
                in_=x_r[:, blk * DBLK:(blk + 1) * DBLK, :])
            return stag

        def transform_block(blk, stag):
            """X~[si, hp(34), j(4), t(16)] bf16 from raw stag [si, 32, 32]."""
            xt = xpool.tile([128, DBLK, HP, NJ, NT], BF16, tag="xt", name=f"xt{blk}")
            v = nc.vector
            g = nc.gpsimd
            # h-pad rows 0 and 33 are transforms of zero rows
            v.memset(xt[:, :, 0, :, :], 0.0)
            v.memset(xt[:, :, HP - 1, :, :], 0.0)
            hp = xt[:, :, 1:R + 1, :, :]
            # j0 = x[2t-1] - x[2t+1]: t=0 -> -x[1]; t 1..15
            v.tensor_scalar_mul(out=hp[:, :, :, 0, 0:1],
                                in0=stag[:, :, :, 1:2], scalar1=-1.0)
            v.tensor_sub(out=hp[:, :, :, 0, 1:16],
                         in0=stag[:, :, :, 1:30:2], in1=stag[:, :, :, 3:32:2])
            # j1 = x[2t] + x[2t+1];  j2 = x[2t+1] - x[2t]
            v.tensor_add(out=hp[:, :, :, 1, :],
                         in0=stag[:, :, :, 0:32:2], in1=stag[:, :, :, 1:32:2])
            g.tensor_sub(out=hp[:, :, :, 2, :],
                         in0=stag[:, :, :, 1:32:2], in1=stag[:, :, :, 0:32:2])
            # j3 = x[2t] - x[2t+2]: t 0..14; t=15 -> x[30]
            g.tensor_sub(out=hp[:, :, :, 3, 0:15],
                         in0=stag[:, :, :, 0:29:2], in1=stag[:, :, :, 2:31:2])
            g.tensor_copy(out=hp[:, :, :, 3, 15:16], in_=stag[:, :, :, 30:31])
            xblocks[blk] = xt

        st0 = stage_block(0)
        st1 = stage_block(1)

        # ---- phase A: styles, winograd-domain bf16 weights, demod scale ----
        with tc.tile_pool(name="ps_a", bufs=2, space="PSUM") as ps_a:
            ident = singles.tile([128, 128], F32)
            make_identity(nc, ident)

            # affine_weight.T, chunked over the 512-dim
            awt = singles.tile([128, W_DIM], F32)
            for c in range(4):
                paw = ps_a.tile([128, 128], F32, tag="paw", name=f"paw{c}")
                nc.tensor.transpose(paw, aw_sb[:, c * 128:(c + 1) * 128], ident)
                nc.vector.tensor_copy(out=awt[:, c * 128:(c + 1) * 128], in_=paw)

            ps_sty = ps_a.tile([128, 1], F32, tag="ps_sty")
            for c in range(4):
                nc.tensor.matmul(ps_sty, lhsT=awt[:, c * 128:(c + 1) * 128],
                                 rhs=wv_sb[:, c:c + 1], start=(c == 0), stop=(c == 3))
            styles = singles.tile([128, 1], F32)
            nc.scalar.activation(out=styles, in_=ps_sty, func=AF.Identity,
                                 bias=ab_sb, scale=1.0 / float(np.sqrt(W_DIM)))
            styles2 = singles.tile([128, 1], F32)
            nc.vector.tensor_mul(out=styles2, in0=styles, in1=styles)
            styles_h = singles.tile([128, 1], F32)
            nc.vector.tensor_scalar_mul(out=styles_h, in0=styles, scalar1=0.5)

            transform_block(0, st0)

            # per (kd,kh) group g: transpose the 3 kw taps, build W~0..3 bf16,
            # square on ScalarE, accumulate sumsq via styles^2-matmul (PE).
            wnat_t = wnat.rearrange("p (ci t) -> p t ci", t=NTAPS)
            wj = singles.tile([128, 9 * NJ * 128], BF16)
            ps_dm = ps_a.tile([128, 1], F32, tag="ps_dm")
            for g in range(9):
                pw = [None] * 3
                for kw in range(3):
                    t = g * 3 + kw
                    ppw = ps_a.tile([128, 128], F32, tag="paw", name=f"pw{t}")
                    nc.tensor.transpose(ppw, wnat_t[:, t, :], ident)
                    pw[kw] = pwpool.tile([128, 128], F32, tag="pw", name=f"pwc{t}")
                    nc.vector.tensor_copy(out=pw[kw], in_=ppw)
                    sq = sqpool.tile([128, 128], F32, tag="sq", name=f"sq{t}")
                    nc.scalar.activation(out=sq, in_=ppw, func=AF.Square)
                    nc.tensor.matmul(ps_dm, lhsT=sq, rhs=styles2,
                                     start=(t == 0), stop=(t == NTAPS - 1))
                base = g * NJ * 128
                wjv = [wj[:, base + j * 128: base + (j + 1) * 128] for j in range(4)]
                nc.vector.tensor_scalar_mul(out=wjv[0], in0=pw[0], scalar1=styles)
                nc.vector.tensor_scalar_mul(out=wjv[3], in0=pw[2], scalar1=styles)
                s01 = tpool.tile([128, 128], F32, tag="s01", name=f"s01_{g}")
                nc.gpsimd.tensor_add(out=s01, in0=pw[0], in1=pw[2])
                u0 = tpool.tile([128, 128], F32, tag="u0", name=f"u0_{g}")
                nc.gpsimd.tensor_add(out=u0, in0=s01, in1=pw[1])
                nc.vector.tensor_scalar_mul(out=wjv[1], in0=u0, scalar1=styles_h)
                u1 = tpool.tile([128, 128], F32, tag="u1", name=f"u1_{g}")
                nc.gpsimd.tensor_sub(out=u1, in0=s01, in1=pw[1])
                nc.vector.tensor_scalar_mul(out=wjv[2], in0=u1, scalar1=styles_h)

            transform_block(1, st1)

            # scale[co] = GAIN * rsqrt(sumsq+EPS)
            eps_sb = singles.tile([128, 1], F32)
            nc.vector.memset(eps_sb, EPS / (GAIN * GAIN))
            sc_tmp = singles.tile([128, 1], F32)
            nc.scalar.activation(out=sc_tmp, in_=ps_dm, func=AF.Sqrt,
                                 bias=eps_sb, scale=1.0 / (GAIN * GAIN))
            scale_sb = singles.tile([128, 1], F32)
            nc.vector.reciprocal(out=scale_sb, in_=sc_tmp)
            bias_g = singles.tile([128, 1], F32)
            nc.scalar.mul(out=bias_g, in_=bs_sb, mul=GAIN)

        pspool = ctx.enter_context(tc.tile_pool(name="pspool", bufs=2, space="PSUM"))

        # ---- phase B: winograd-domain conv, one d-slice per psum group ----
        next_blk = 2
        for d in range(R):
            while next_blk < NBLK and (d + 1) // DBLK + 1 >= next_blk:
                transform_block(next_blk, stage_block(next_blk))
                next_blk += 1

            kds = [kd for kd in range(3) if 0 <= d + kd - 1 < R]
            gs = [kd * 3 + kh for kd in kds for kh in range(3)]
            ps = [pspool.tile([128, R, NT], F32, tag=f"ps{j}", name=f"ps{d}_{j}")
                  for j in range(NJ)]

            for gi, g in enumerate(gs):
                kd, kh = g // 3, g % 3
                s = d + kd - 1
                xt = xblocks[s // DBLK]
                si = s % DBLK
                for j in range(NJ):
                    nc.tensor.matmul(
                        ps[j], lhsT=wj[:, (g * NJ + j) * 128:(g * NJ + j + 1) * 128],
                        rhs=xt[:, si, kh:kh + R, j, :],
                        start=(gi == 0), stop=(gi == len(gs) - 1))

            # inverse transform + fused prelu epilogue + store
            # TensorTensor may read only one PSUM operand: stage m1/m2 in SBUF.
            ob = obpool.tile([128, R, R], F32, tag="ob", name=f"ob{d}")
            s1 = tpool.tile([128, R, NT], F32, tag="s1", name=f"s1_{d}")
            s2 = tpool.tile([128, R, NT], F32, tag="s2", name=f"s2_{d}")
            t_e = tpool.tile([128, R, NT], F32, tag="t_e", name=f"te{d}")
            t_e2 = tpool.tile([128, R, NT], F32, tag="t_e2", name=f"te2{d}")
            t_o = tpool.tile([128, R, NT], F32, tag="t_o", name=f"to{d}")
            t_o2 = tpool.tile([128, R, NT], F32, tag="t_o2", name=f"to2{d}")
            nc.scalar.activation(out=s1, in_=ps[1], func=AF.Identity)
            nc.scalar.activation(out=s2, in_=ps[2], func=AF.Identity)
            nc.vector.tensor_add(out=t_e, in0=ps[0], in1=s1)
            nc.gpsimd.tensor_add(out=t_e2, in0=t_e, in1=s2)
            nc.gpsimd.tensor_sub(out=t_o, in0=s1, in1=s2)
            nc.vector.tensor_sub(out=t_o2, in0=t_o, in1=ps[3])
            nc.scalar.activation(out=ob[:, :, 0::2], in_=t_e2, func=AF.Prelu,
                                 bias=bias_g, scale=scale_sb, alpha=SLOPE)
            nc.scalar.activation(out=ob[:, :, 1::2], in_=t_o2, func=AF.Prelu,
                                 bias=bias_g, scale=scale_sb, alpha=SLOPE)
            nc.sync.dma_start(out=out_d[:, d * 1024:(d + 1) * 1024],
                              in_=ob.rearrange("p h w -> p (h w)"))

    nc.compile()
    return nc


def kernel(**inputs):
    x = np.ascontiguousarray(np.asarray(inputs["x"], dtype=np.float32))
    w = np.ascontiguousarray(np.asarray(inputs["w"], dtype=np.float32))
    weight = np.ascontiguousarray(np.asarray(inputs["weight"], dtype=np.float32))
    aw = np.ascontiguousarray(np.asarray(inputs["affine_weight"], dtype=np.float32))
    ab = np.ascontiguousarray(np.asarray(inputs["affine_bias"], dtype=np.float32))
    bias = np.ascontiguousarray(np.asarray(inputs["bias"], dtype=np.float32))

    if "nc" not in _cache:
        _cache["nc"] = _build()
    nc = _cache["nc"]

    wt2 = weight.reshape(COUT, CIN * NTAPS)
    in_maps = [
        {
            "x": x[b].reshape(CIN, R * R * R),
            "wvec": w[b],
            "weight": wt2,
            "aw": aw,
            "ab": ab,
            "bias": bias,
        }
        for b in range(B)
    ]
    res = run_bass_kernel_spmd(nc, in_maps, list(range(NCORES)))
    out = np.stack([res.results[b]["out"].reshape(COUT, R, R, R) for b in range(B)])
    return out.astype(np.float32)


def run_traced(**inputs):
    """Like kernel(), but also returns the profiled HW exec time in ns."""
    x = np.asarray(inputs["x"], dtype=np.float32)
    w = np.asarray(inputs["w"], dtype=np.float32)
    weight = np.asarray(inputs["weight"], dtype=np.float32)
    aw = np.asarray(inputs["affine_weight"], dtype=np.float32)
    ab = np.asarray(inputs["affine_bias"], dtype=np.float32)
    bias = np.asarray(inputs["bias"], dtype=np.float32)
    if "nc" not in _cache:
        _cache["nc"] = _build()
    nc = _cache["nc"]
    wt2 = weight.reshape(COUT, CIN * NTAPS)
    in_maps = [
        {"x": x[b].reshape(CIN, R * R * R), "wvec": w[b], "weight": wt2,
         "aw": aw, "ab": ab, "bias": bias}
        for b in range(B)
    ]
    res = run_bass_kernel_spmd(nc, in_maps, list(range(NCORES)), trace=True)
    out = np.stack([res.results[b]["out"].reshape(COUT, R, R, R) for b in range(B)])
    return out.astype(np.float32), res.exec_time_ns, res
